# revision 1
# baseline (speedup 1.0000x reference)
"""Contrastive L2 loss (match/non-match descriptor loss) on Trainium2.

Strategy: data-parallel over batch B=8 across 8 NeuronCores (1 image pair per
core).  Per core the kernel:
  * loads the index tensors into single-partition SBUF rows,
  * indirect-DMA gathers the 64B descriptor rows for match (2x5000) and
    non-match (2x50000) indices straight from the HBM-resident outA/outB
    shards (this random 64B-row gather is the memory roofline of the op),
  * computes sum((mA-mB)^2), dist=sqrt(sum((nA-nB)^2, D)), sum(dist),
    nm=relu(mean-dist)^2 sums and the hard-negative count on DVE/ACT,
  * writes 4 partial scalars to DRAM.
Host combines the 8x4 partials into the 3 reference scalars.

Indirect-DMA contract (validated on HW): the index AP must lower to a 1-D
step-1 element stream (shape [1, n], one SBUF partition row); descriptor k
reads index word k and writes the k-th [16-element] inner block of the dest
AP walk, so dest slot (p, j) of a [125, n/125, 16] dest gets row idx[p*(n/125)+j].
A [125, n] index tile does NOT work on hardware even though CoreSim accepts it.

125 partitions are used so 5000 and 50000 both divide evenly -> no padding.
Cross-partition folds go through a tiny SBUF->SBUF reshaping DMA + DVE
reduce (exact fp32; PE is avoided entirely).
"""

import numpy as np

B, N, D = 8, 307200, 16
M, K = 5000, 50000
NON_MATCH_LOSS_WEIGHT = 1.0

P = 125            # partitions used; divides both M and K evenly
MN = M // P        # 40 match rows per partition
KN = K // P        # 400 non-match rows per partition
K_CHUNKS = 5
KC = KN // K_CHUNKS  # rows per partition per chunk (100)
KCHUNK = K // K_CHUNKS  # indices per chunk (12500 descriptors/instruction)

_CACHE = {}


def _build_nc(debug=False):
    import concourse.bacc as bacc
    import concourse.mybir as mybir
    from concourse.bass import AP, IndirectOffsetOnAxis
    from concourse.tile import TileContext


    f32 = mybir.dt.float32
    i32 = mybir.dt.int32
    X = mybir.AxisListType.X
    Alu = mybir.AluOpType
    Act = mybir.ActivationFunctionType

    nc = bacc.Bacc()
    outA = nc.declare_dram_parameter("outA", [N, D], f32, isOutput=False)
    outB = nc.declare_dram_parameter("outB", [N, D], f32, isOutput=False)
    matchA = nc.declare_dram_parameter("matchA", [M], i32, isOutput=False)
    matchB = nc.declare_dram_parameter("matchB", [M], i32, isOutput=False)
    nonMatchA = nc.declare_dram_parameter("nonMatchA", [K], i32, isOutput=False)
    nonMatchB = nc.declare_dram_parameter("nonMatchB", [K], i32, isOutput=False)
    stats = nc.declare_dram_parameter("stats", [1, 4], f32, isOutput=True)
    if debug:
        mA_d = nc.declare_dram_parameter("mA_d", [P, MN * D], f32, isOutput=True)
        mB_d = nc.declare_dram_parameter("mB_d", [P, MN * D], f32, isOutput=True)
        dist_d = nc.declare_dram_parameter("dist_d", [P, KN], f32, isOutput=True)
        parts_d = nc.declare_dram_parameter("parts_d", [P, 4], f32, isOutput=True)
        mean_d = nc.declare_dram_parameter("mean_d", [P, 1], f32, isOutput=True)

    with TileContext(nc) as tc:
        with (
            tc.tile_pool(name="idx", bufs=1) as idxp,
            tc.tile_pool(name="gather", bufs=2) as gp,
            tc.tile_pool(name="work", bufs=2) as wp,
            tc.tile_pool(name="persist", bufs=1) as pp,
        ):
            # ---- index tensors into SBUF, [P, rowlen] ---------------
            midxA = idxp.tile([P, MN], i32)
            midxB = idxp.tile([P, MN], i32)
            nidxA = idxp.tile([P, KN], i32)
            nidxB = idxp.tile([P, KN], i32)
            nc.sync.dma_start(out=midxA[:], in_=matchA[:].rearrange("(p n) -> p n", p=P))
            nc.sync.dma_start(out=midxB[:], in_=matchB[:].rearrange("(p n) -> p n", p=P))
            nc.sync.dma_start(out=nidxA[:], in_=nonMatchA[:].rearrange("(p n) -> p n", p=P))
            nc.sync.dma_start(out=nidxB[:], in_=nonMatchB[:].rearrange("(p n) -> p n", p=P))

            def gather_cols(dst_tile, table, idx_tile, j0, ncols):
                # production-proven shape: idx [P,1] column, dest [P,16] row
                # per instruction (one descriptor per partition).
                for j in range(j0, j0 + ncols):
                    nc.gpsimd.indirect_dma_start(
                        out=dst_tile[:, (j - j0) * D:(j - j0 + 1) * D],
                        out_offset=None, in_=table,
                        in_offset=IndirectOffsetOnAxis(
                            ap=idx_tile[:, j:j + 1], axis=0))

            # ---- persistent accumulators ------------------------------
            dist = pp.tile([P, KN], f32)          # all non-match distances
            parts = pp.tile([P, 4], f32)          # [match_sq, nm_sum, hn, dist_sum]

            # ---- match part ------------------------------------------
            mA = gp.tile([P, MN * D], f32)
            mB = gp.tile([P, MN * D], f32)
            gather_cols(mA, outA[:], midxA, 0, MN)
            gather_cols(mB, outB[:], midxB, 0, MN)
            mD = wp.tile([P, MN * D], f32, bufs=1)
            nc.vector.tensor_sub(mD[:], mA[:], mB[:])
            mSq = wp.tile([P, MN * D], f32, bufs=1)
            nc.vector.tensor_mul(mSq[:], mD[:], mD[:])
            nc.vector.reduce_sum(out=parts[:, 0:1], in_=mSq[:], axis=X)

            # ---- non-match distances, chunked ------------------------
            for c in range(K_CHUNKS):
                sl = slice(c * KC, (c + 1) * KC)
                nA = gp.tile([P, KC * D], f32, tag="nA")
                nB = gp.tile([P, KC * D], f32, tag="nB")
                gather_cols(nA, outA[:], nidxA, c * KC, KC)
                gather_cols(nB, outB[:], nidxB, c * KC, KC)
                df = wp.tile([P, KC * D], f32, tag="df")
                nc.vector.tensor_sub(df[:], nA[:], nB[:])
                sq = wp.tile([P, KC * D], f32, tag="sq")
                nc.vector.tensor_mul(sq[:], df[:], df[:])
                d2 = wp.tile([P, KC], f32, tag="d2")
                nc.vector.reduce_sum(
                    out=d2[:], in_=sq[:].rearrange("p (n d) -> p n d", d=D), axis=X)
                nc.scalar.activation(out=dist[:, sl], in_=d2[:], func=Act.Sqrt)

            # ---- mean over all K distances ---------------------------
            # per-partition row sums, then fold partitions via a tiny
            # SBUF->SBUF reshaping DMA + DVE reduce (exact fp32).
            nc.vector.reduce_sum(out=parts[:, 3:4], in_=dist[:], axis=X)
            dcol = pp.tile([1, P], f32)
            nc.sync.dma_start(out=dcol[:], in_=parts[:, 3:4])
            meanp0 = pp.tile([1, 1], f32)
            nc.vector.reduce_sum(out=meanp0[:], in_=dcol[:], axis=X)
            nc.scalar.mul(meanp0[:], meanp0[:], 1.0 / K)
            mean_bc = pp.tile([P, 1], f32)
            nc.gpsimd.partition_broadcast(out_ap=mean_bc[:], in_ap=meanp0[:])

            # ---- nm = relu(mean - dist)^2, hn = count(dist < mean) ---
            t = wp.tile([P, KN], f32, bufs=1)
            nc.vector.tensor_scalar(
                out=t[:], in0=dist[:], scalar1=mean_bc[:, 0:1], scalar2=None,
                op0=Alu.subtract)          # t = dist - mean
            nm = wp.tile([P, KN], f32, bufs=1)
            # (t min 0) * t == relu(mean-dist)^2 elementwise
            nc.vector.scalar_tensor_tensor(
                out=nm[:], in0=t[:], scalar=0.0, in1=t[:],
                op0=Alu.min, op1=Alu.mult)
            nc.vector.reduce_sum(out=parts[:, 1:2], in_=nm[:], axis=X)
            ind = wp.tile([P, KN], f32, bufs=1)
            nc.vector.tensor_scalar(
                out=ind[:], in0=t[:], scalar1=0.0, scalar2=None,
                op0=Alu.is_lt)
            nc.vector.reduce_sum(out=parts[:, 2:3], in_=ind[:], axis=X)

            # ---- fold partitions and write out -----------------------
            prow = pp.tile([1, P * 4], f32)
            nc.sync.dma_start(out=prow[:], in_=parts[:])
            stats_row = pp.tile([1, 4], f32)
            # prow layout is partition-major: element [p*4 + c]; view as
            # [1, 4(c, step 1), 125(p, step 4)] and reduce the p axis.
            nc.vector.reduce_sum(
                out=stats_row[:],
                in_=prow[:].rearrange("o (p c) -> o c p", c=4),
                axis=X)
            nc.sync.dma_start(out=stats[:], in_=stats_row[:])

            if debug:
                nc.sync.dma_start(out=mA_d[:], in_=mA[:])
                nc.sync.dma_start(out=mB_d[:], in_=mB[:])
                nc.sync.dma_start(out=dist_d[:], in_=dist[:])
                nc.sync.dma_start(out=parts_d[:], in_=parts[:])
                nc.sync.dma_start(out=mean_d[:], in_=mean_bc[:])

    nc.finalize()
    return nc


def _get_nc():
    if "nc" not in _CACHE:
        _CACHE["nc"] = _build_nc()
    return _CACHE["nc"]


def kernel(outA, outB, matchA, matchB, nonMatchA, nonMatchB, hardNegative):
    from concourse.bass_utils import run_bass_kernel_spmd

    outA = np.asarray(outA, dtype=np.float32)
    outB = np.asarray(outB, dtype=np.float32)
    matchA = np.asarray(matchA, dtype=np.int32)
    matchB = np.asarray(matchB, dtype=np.int32)
    nonMatchA = np.asarray(nonMatchA, dtype=np.int32)
    nonMatchB = np.asarray(nonMatchB, dtype=np.int32)
    hard = int(np.asarray(hardNegative))

    nc = _get_nc()
    in_maps = [
        {
            "outA": np.ascontiguousarray(outA[b]),
            "outB": np.ascontiguousarray(outB[b]),
            "matchA": np.ascontiguousarray(matchA[b]),
            "matchB": np.ascontiguousarray(matchB[b]),
            "nonMatchA": np.ascontiguousarray(nonMatchA[b]),
            "nonMatchB": np.ascontiguousarray(nonMatchB[b]),
        }
        for b in range(B)
    ]
    res = run_bass_kernel_spmd(nc, in_maps, core_ids=list(range(B)))
    stats = np.stack([np.asarray(r["stats"]).reshape(4) for r in res.results])

    match_sq = stats[:, 0].astype(np.float64)
    nm_sum = stats[:, 1].astype(np.float64)
    hn = stats[:, 2].astype(np.float64)
    match_loss = (match_sq / M).astype(np.float32)
    if hard:
        denom = np.where(hn == 0, float(K), hn)
    else:
        denom = np.full(B, float(K))
    nm_loss = (NON_MATCH_LOSS_WEIGHT * nm_sum / denom).astype(np.float32)

    match_sum = np.float32(np.sum(match_loss, dtype=np.float32))
    non_match_sum = np.float32(np.sum(nm_loss, dtype=np.float32))
    return (
        np.float32(match_sum + non_match_sum),
        match_sum,
        non_match_sum,
    )



# revision 14
# speedup vs baseline: 23.3099x; 23.3099x over previous
"""Contrastive L2 loss (match/non-match descriptor loss) on Trainium2.

Strategy: data-parallel over batch B=8 across 8 NeuronCores (1 image pair per
core).  Per core the kernel:
  * loads the index tensors into [128, E] SBUF tiles,
  * indirect-DMA gathers the 64B descriptor rows for match (2x5120) and
    non-match (2x50048, 7 chunks/side) straight from the HBM-resident
    outA/outB shards (this random 64B-row gather is the memory roofline),
  * computes sum((mA-mB)^2), dist=sqrt(sum((nA-nB)^2, D)), sum(dist),
    nm=relu(mean-dist)^2 sums and the hard-negative count on DVE/ACT,
  * writes 4 partial scalars to DRAM.
Host combines the 8x4 partials into the 3 reference scalars.

Indirect-DMA contract (validated on HW via a mapping probe, confirmed by the
SWDGE ucode dma_memcopy.cpp): the offset AP must be a [128, E] tile (the Q7
allgather always reads all 128 channels at the AP's per-partition byte
offset); the descriptor fed by idx[p, j] lands at dest partition p, 16-elem
block j.  Max 8192 indices per instruction (Q7 data-scratch limit: 512
vectors x 16 lanes).  Per-instruction Pool time is ~1.15us FLAT in n
(measured 1024..8192), so few big gathers >> many small ones: the previous
125-descriptor-per-instruction version spent 1.43ms in 1080 instructions.

Padding: gather counts are rounded up to multiples of 128.  Pad slots use
index 0 (safe reads) and are laid out so they are excluded purely by AP
slicing with partition start 0 (engine APs must start on an aligned
partition, so no [127:128, ...]-style writes are allowed):
  * match: 5120 slots = [128, 40]; real 5000 = partitions 0..124 exactly;
    the match reduction reads [0:125, :].
  * non-match: 50048 slots = [128, 391] in 7 chunks (6x56 + 55 cols); real
    50000 = everything except partition 127, cols 343..391 (48 slots).
    dist/nm/ind are computed over the full [128, 391] (pad values are
    finite garbage) and every reduction splits into two column groups:
    cols 0:343 over all 128 partitions + cols 343:391 over partitions
    0:127, accumulated into separate `parts` columns and summed in the
    final fold.
"""

import numpy as np

B, N, D = 8, 307200, 16
M, K = 5000, 50000
NON_MATCH_LOSS_WEIGHT = 1.0

EM = 40                      # match index columns; 128*40 = 5120 >= M
KE = [56, 56, 56, 56, 56, 56, 55]   # non-match chunk widths (cols)
KCOLS = sum(KE)              # 391; 128*391 = 50048 >= K
KOFF = [sum(KE[:i]) for i in range(len(KE))]
# last chunk: real slots = partitions 0..126 full + partition 127 first 7 cols
LAST_E = KE[-1]
LAST_OFF = KOFF[-1]
LAST_P127_REAL = 7           # (K - 6*128*56) - 127*55 = 6992 - 6985
GSPLIT = LAST_OFF + LAST_P127_REAL  # 343: cols < GSPLIT are real on all 128
# parts columns: 0=match_sq, 1=nm_a, 2=hn_a, 3=dist_a, 4=dist_b, 5=nm_b,
# 6=hn_b, 7=unused   (a: cols 0:GSPLIT x 128p, b: cols GSPLIT: x 127p)
PC = 8

_CACHE = {}


def _build_nc():
    import concourse.bacc as bacc
    import concourse.mybir as mybir
    from concourse.bass import IndirectOffsetOnAxis
    from concourse.tile import TileContext

    f32 = mybir.dt.float32
    i32 = mybir.dt.int32
    X = mybir.AxisListType.X
    Alu = mybir.AluOpType
    Act = mybir.ActivationFunctionType

    nc = bacc.Bacc()
    outA = nc.declare_dram_parameter("outA", [N, D], f32, isOutput=False)
    outB = nc.declare_dram_parameter("outB", [N, D], f32, isOutput=False)
    idxMA = nc.declare_dram_parameter("idxMA", [128 * EM], i32, isOutput=False)
    idxMB = nc.declare_dram_parameter("idxMB", [128 * EM], i32, isOutput=False)
    idxNA = nc.declare_dram_parameter("idxNA", [128 * KCOLS], i32, isOutput=False)
    idxNB = nc.declare_dram_parameter("idxNB", [128 * KCOLS], i32, isOutput=False)
    stats = nc.declare_dram_parameter("stats", [1, PC], f32, isOutput=True)

    with TileContext(nc) as tc:
        with (
            tc.tile_pool(name="idx", bufs=1) as idxp,
            tc.tile_pool(name="gather", bufs=2) as gp,
            tc.tile_pool(name="work", bufs=2) as wp,
            tc.tile_pool(name="persist", bufs=1) as pp,
        ):
            # ---- index tiles ----------------------------------------
            tMA = idxp.tile([128, EM], i32)
            tMB = idxp.tile([128, EM], i32)
            tNA = idxp.tile([128, KCOLS], i32)
            tNB = idxp.tile([128, KCOLS], i32)
            nc.sync.dma_start(out=tMA[:], in_=idxMA[:].rearrange("(p e) -> p e", e=EM))
            nc.sync.dma_start(out=tMB[:], in_=idxMB[:].rearrange("(p e) -> p e", e=EM))
            nc.sync.dma_start(out=tNA[:], in_=idxNA[:].rearrange("(p e) -> p e", e=KCOLS))
            nc.sync.dma_start(out=tNB[:], in_=idxNB[:].rearrange("(p e) -> p e", e=KCOLS))

            # ---- persistent accumulators ----------------------------
            dist = pp.tile([128, KCOLS], f32)
            parts = pp.tile([128, PC], f32)
            nc.vector.memset(parts[:], 0.0)

            def gather(dst, table, idx_ap):
                nc.gpsimd.indirect_dma_start(
                    out=dst, out_offset=None, in_=table,
                    in_offset=IndirectOffsetOnAxis(ap=idx_ap, axis=0))

            # ---- match part -----------------------------------------
            mA = gp.tile([128, EM * D], f32)
            mB = gp.tile([128, EM * D], f32)
            gather(mA[:], outA[:], tMA[:])
            gather(mB[:], outB[:], tMB[:])
            mD = wp.tile([128, EM * D], f32, bufs=1)
            nc.vector.tensor_sub(mD[:], mA[:], mB[:])
            mSq = wp.tile([128, EM * D], f32, bufs=1)
            # ACT: square + free-dim accumulate in one pass; real rows only.
            nc.scalar.activation(
                out=mSq[0:125, :], in_=mD[0:125, :], func=Act.Square,
                accum_out=parts[0:125, 0:1])

            # ---- non-match distances, chunked -----------------------
            for c, (off, E) in enumerate(zip(KOFF, KE)):
                nA = gp.tile([128, E * D], f32, tag="nA")
                nB = gp.tile([128, E * D], f32, tag="nB")
                gather(nA[:], outA[:], tNA[:, off:off + E])
                gather(nB[:], outB[:], tNB[:, off:off + E])
                df = wp.tile([128, E * D], f32, tag="df")
                nc.vector.tensor_sub(df[:], nA[:], nB[:])
                sq = wp.tile([128, E * D], f32, tag="sq")
                nc.scalar.square(sq[:], df[:])
                d2 = wp.tile([128, E], f32, tag="d2")
                nc.vector.reduce_sum(
                    out=d2[:], in_=sq[:].rearrange("p (n d) -> p n d", d=D), axis=X)
                nc.scalar.activation(out=dist[:, off:off + E], in_=d2[:],
                                     func=Act.Sqrt)

            # ---- mean over all K distances --------------------------
            # column-group split keeps pad slots (p127, cols GSPLIT:) out
            nc.vector.reduce_sum(out=parts[:, 3:4], in_=dist[:, 0:GSPLIT], axis=X)
            nc.vector.reduce_sum(out=parts[0:127, 4:5], in_=dist[0:127, GSPLIT:],
                                 axis=X)
            dcol = pp.tile([1, 256], f32)
            nc.sync.dma_start(out=dcol[:], in_=parts[:, 3:5])
            meanp0 = pp.tile([1, 1], f32)
            nc.vector.reduce_sum(out=meanp0[:], in_=dcol[:], axis=X)
            nc.scalar.mul(meanp0[:], meanp0[:], 1.0 / K)
            mean_bc = pp.tile([128, 1], f32)
            nc.gpsimd.partition_broadcast(out_ap=mean_bc[:], in_ap=meanp0[:])

            # ---- nm = relu(mean - dist)^2, hn = count(dist < mean) --
            t = wp.tile([128, KCOLS], f32, bufs=1)
            nc.vector.tensor_scalar(
                out=t[:], in0=dist[:], scalar1=mean_bc[:, 0:1], scalar2=None,
                op0=Alu.subtract)          # t = dist - mean
            nm = wp.tile([128, KCOLS], f32, bufs=1)
            nc.vector.scalar_tensor_tensor(
                out=nm[:], in0=t[:], scalar=0.0, in1=t[:],
                op0=Alu.min, op1=Alu.mult)  # min(t,0)*t == relu(mean-dist)^2
            nc.vector.reduce_sum(out=parts[:, 1:2], in_=nm[:, 0:GSPLIT], axis=X)
            nc.vector.reduce_sum(out=parts[0:127, 5:6], in_=nm[0:127, GSPLIT:],
                                 axis=X)
            ind = wp.tile([128, KCOLS], f32, bufs=1)
            nc.vector.tensor_scalar(
                out=ind[:], in0=t[:], scalar1=0.0, scalar2=None,
                op0=Alu.is_lt)
            nc.vector.reduce_sum(out=parts[:, 2:3], in_=ind[:, 0:GSPLIT], axis=X)
            nc.vector.reduce_sum(out=parts[0:127, 6:7], in_=ind[0:127, GSPLIT:],
                                 axis=X)

            # ---- fold partitions and write out ----------------------
            prow = pp.tile([1, 128 * PC], f32)
            nc.sync.dma_start(out=prow[:], in_=parts[:])
            stats_row = pp.tile([1, PC], f32)
            nc.vector.reduce_sum(
                out=stats_row[:],
                in_=prow[:].rearrange("o (p c) -> o c p", c=PC),
                axis=X)
            nc.sync.dma_start(out=stats[:], in_=stats_row[:])

    nc.finalize()
    return nc


def _get_nc():
    if "nc" not in _CACHE:
        _CACHE["nc"] = _build_nc()
    return _CACHE["nc"]


def _marshal_match(idx):
    """[M] int32 -> [128*EM] row-major [128, EM]; pads (idx 0) fill
    partitions 125..127."""
    buf = np.zeros(128 * EM, dtype=np.int32)
    buf[:M] = idx
    return buf


def _marshal_nonmatch(idx):
    """[K] int32 -> [128*KCOLS] laid out as [128, KCOLS] where chunk c's
    columns hold its 128*E_c indices row-major; pads (idx 0) fall on
    partition 127, cols LAST_OFF+7.. of the last chunk."""
    padded = np.zeros(128 * KCOLS, dtype=np.int32)
    padded[:K] = idx
    full = np.empty((128, KCOLS), dtype=np.int32)
    base = 0
    for off, E in zip(KOFF, KE):
        nblk = 128 * E
        full[:, off:off + E] = padded[base:base + nblk].reshape(128, E)
        base += nblk
    return full.reshape(-1)


def _make_in_maps(outA, outB, matchA, matchB, nonMatchA, nonMatchB):
    outA = np.asarray(outA, dtype=np.float32)
    outB = np.asarray(outB, dtype=np.float32)
    matchA = np.asarray(matchA, dtype=np.int32)
    matchB = np.asarray(matchB, dtype=np.int32)
    nonMatchA = np.asarray(nonMatchA, dtype=np.int32)
    nonMatchB = np.asarray(nonMatchB, dtype=np.int32)
    return [
        {
            "outA": np.ascontiguousarray(outA[b]),
            "outB": np.ascontiguousarray(outB[b]),
            "idxMA": _marshal_match(matchA[b]),
            "idxMB": _marshal_match(matchB[b]),
            "idxNA": _marshal_nonmatch(nonMatchA[b]),
            "idxNB": _marshal_nonmatch(nonMatchB[b]),
        }
        for b in range(B)
    ]


def kernel(outA, outB, matchA, matchB, nonMatchA, nonMatchB, hardNegative):
    from concourse.bass_utils import run_bass_kernel_spmd

    hard = int(np.asarray(hardNegative))
    nc = _get_nc()
    in_maps = _make_in_maps(outA, outB, matchA, matchB, nonMatchA, nonMatchB)
    res = run_bass_kernel_spmd(nc, in_maps, core_ids=list(range(B)))
    stats = np.stack([np.asarray(r["stats"]).reshape(PC) for r in res.results])

    match_sq = stats[:, 0].astype(np.float64)
    nm_sum = (stats[:, 1] + stats[:, 5]).astype(np.float64)
    hn = (stats[:, 2] + stats[:, 6]).astype(np.float64)
    match_loss = (match_sq / M).astype(np.float32)
    if hard:
        denom = np.where(hn == 0, float(K), hn)
    else:
        denom = np.full(B, float(K))
    nm_loss = (NON_MATCH_LOSS_WEIGHT * nm_sum / denom).astype(np.float32)

    match_sum = np.float32(np.sum(match_loss, dtype=np.float32))
    non_match_sum = np.float32(np.sum(nm_loss, dtype=np.float32))
    return (
        np.float32(match_sum + non_match_sum),
        match_sum,
        non_match_sum,
    )


# revision 15
# speedup vs baseline: 25.3255x; 1.0865x over previous
"""Contrastive L2 loss (match/non-match descriptor loss) on Trainium2.

Strategy: data-parallel over batch B=8 across 8 NeuronCores (1 image pair per
core).  Per core the kernel:
  * loads the index tensors into [128, E] SBUF tiles (two HWDGE rings),
  * indirect-DMA gathers the 64B descriptor rows for match (2x5120) and
    non-match (2x50048, 7 chunks/side) straight from the HBM-resident
    outA/outB shards (this random 64B-row gather is the memory roofline),
  * computes sum((mA-mB)^2) (DVE, exact), dist=sqrt(sum((nA-nB)^2, D))
    (DVE squares + ACT sqrt), mean via a ones-matmul on the idle PE (fused
    cross-partition reduce + broadcast through PSUM), then
    nm=relu(mean-dist)^2 sums and the hard-negative count on DVE,
  * folds the per-partition partials with a second ones-matmul and writes
    one [1, PC] stats row to DRAM.
Host combines the 8 stats rows into the 3 reference scalars.

Indirect-DMA contract (validated on HW via a mapping probe, confirmed in the
SWDGE ucode dma_memcopy.cpp): the offset AP must be a [128, E] tile (the Q7
allgather always reads all 128 channels at the AP's per-partition byte
offset); the descriptor fed by idx[p, j] lands at dest partition p, 16-elem
block j.  Max 8192 indices per instruction (Q7 data-scratch: 512 vectors).
Per-instruction Pool time is ~1.1us FLAT in n (measured 1024..8192), so few
big gathers >> many small ones: the previous 125-descriptor-per-instruction
version spent 1.43ms in 1080 instructions; this one spends ~18us in 16.

Padding: gather counts are rounded up to multiples of 128.  Pad slots use
index 0 (safe reads) and are excluded purely by AP slicing with partition
start 0 (engine APs must start on an aligned partition, so no
[127:128, ...]-style accesses):
  * match: 5120 slots = [128, 40]; real 5000 = partitions 0..124 exactly;
    the match reduction reads [0:125, :].
  * non-match: 50048 slots = [128, 391] in 7 chunks (6x64 + 7 cols); the
    first 6 chunks are fully real; the last chunk (cols 384:391) has real
    slots on partitions 0..120 everywhere plus partition 121 in its first
    column.  dist/nm/ind are computed over the full [128, 391] (pad values
    are finite garbage) and every reduction splits into three column
    groups: cols 0:384 x 128p, col 384 x 122p, cols 385:391 x 121p, into
    separate `parts` columns summed in the final fold.
"""

import numpy as np

B, N, D = 8, 307200, 16
M, K = 5000, 50000
NON_MATCH_LOSS_WEIGHT = 1.0

EM = 40                      # match index columns; 128*40 = 5120 >= M
KE = [64, 64, 64, 64, 64, 64, 7]    # non-match chunk widths (cols)
KCOLS = sum(KE)              # 391; 128*391 = 50048 >= K
KOFF = [sum(KE[:i]) for i in range(len(KE))]
LAST_E = KE[-1]
LAST_OFF = KOFF[-1]          # 384
_R = K - 128 * LAST_OFF      # real slots in last chunk: 848
PFULL = _R // LAST_E         # 121 partitions fully real in last chunk
PREM = _R % LAST_E           # 1 extra real col on partition PFULL
assert PREM > 0
# parts columns: 0=match_sq, 1..3=nm_{a,b,c}, 4..6=hn_{a,b,c},
# 7..9=dist_{a,b,c}; groups: a=cols 0:384 x 128p, b=col 384 x (PFULL+1)p,
# c=cols 385:391 x PFULL p
PC = 10

_CACHE = {}


def _build_nc():
    import concourse.bacc as bacc
    import concourse.mybir as mybir
    from concourse.bass import IndirectOffsetOnAxis, MemorySpace
    from concourse.tile import TileContext

    f32 = mybir.dt.float32
    i32 = mybir.dt.int32
    X = mybir.AxisListType.X
    Alu = mybir.AluOpType
    Act = mybir.ActivationFunctionType

    nc = bacc.Bacc()
    outA = nc.declare_dram_parameter("outA", [N, D], f32, isOutput=False)
    outB = nc.declare_dram_parameter("outB", [N, D], f32, isOutput=False)
    idxMA = nc.declare_dram_parameter("idxMA", [128 * EM], i32, isOutput=False)
    idxMB = nc.declare_dram_parameter("idxMB", [128 * EM], i32, isOutput=False)
    idxNA = nc.declare_dram_parameter("idxNA", [128 * KCOLS], i32, isOutput=False)
    idxNB = nc.declare_dram_parameter("idxNB", [128 * KCOLS], i32, isOutput=False)
    stats = nc.declare_dram_parameter("stats", [1, PC], f32, isOutput=True)

    with TileContext(nc) as tc:
        with (
            tc.tile_pool(name="idx", bufs=1) as idxp,
            tc.tile_pool(name="gather", bufs=2) as gp,
            tc.tile_pool(name="work", bufs=2) as wp,
            tc.tile_pool(name="persist", bufs=1) as pp,
            tc.tile_pool(name="psum", bufs=1, space=MemorySpace.PSUM) as psp,
        ):
            # ---- index tiles (split across the two HWDGE rings) -----
            tMA = idxp.tile([128, EM], i32)
            tMB = idxp.tile([128, EM], i32)
            tNA = idxp.tile([128, KCOLS], i32)
            tNB = idxp.tile([128, KCOLS], i32)
            nc.sync.dma_start(out=tMA[:], in_=idxMA[:].rearrange("(p e) -> p e", e=EM))
            nc.scalar.dma_start(out=tMB[:], in_=idxMB[:].rearrange("(p e) -> p e", e=EM))
            nc.sync.dma_start(out=tNA[:], in_=idxNA[:].rearrange("(p e) -> p e", e=KCOLS))
            nc.scalar.dma_start(out=tNB[:], in_=idxNB[:].rearrange("(p e) -> p e", e=KCOLS))

            # ---- persistent state -----------------------------------
            dist = pp.tile([128, KCOLS], f32)
            parts = pp.tile([128, PC], f32)
            ones = pp.tile([128, 128], f32)
            nc.vector.memset(parts[:], 0.0)
            nc.vector.memset(ones[:], 1.0)

            def gather(dst, table, idx_ap):
                nc.gpsimd.indirect_dma_start(
                    out=dst, out_offset=None, in_=table,
                    in_offset=IndirectOffsetOnAxis(ap=idx_ap, axis=0))

            # ---- match part -----------------------------------------
            mA = gp.tile([128, EM * D], f32)
            mB = gp.tile([128, EM * D], f32)
            gather(mA[:], outA[:], tMA[:])
            gather(mB[:], outB[:], tMB[:])
            mD = wp.tile([128, EM * D], f32, bufs=1)
            nc.vector.tensor_sub(mD[:], mA[:], mB[:])
            mSq = wp.tile([128, EM * D], f32, bufs=1)
            # fused exact square + free-dim sum on DVE (accum_out=sum(out))
            nc.vector.scalar_tensor_tensor(
                out=mSq[0:125, :], in0=mD[0:125, :], scalar=0.0,
                in1=mD[0:125, :], op0=Alu.add, op1=Alu.mult,
                accum_out=parts[0:125, 0:1])

            # ---- non-match distances, chunked -----------------------
            for c, (off, E) in enumerate(zip(KOFF, KE)):
                nA = gp.tile([128, E * D], f32, tag=f"nA{E}")
                nB = gp.tile([128, E * D], f32, tag=f"nB{E}")
                gather(nA[:], outA[:], tNA[:, off:off + E])
                gather(nB[:], outB[:], tNB[:, off:off + E])
                df = wp.tile([128, E * D], f32, tag=f"df{E}")
                nc.vector.tensor_sub(df[:], nA[:], nB[:])
                sq = wp.tile([128, E * D], f32, tag=f"sq{E}")
                nc.vector.tensor_mul(sq[:], df[:], df[:])
                d2 = wp.tile([128, E], f32, tag=f"d2{E}")
                nc.vector.reduce_sum(
                    out=d2[:], in_=sq[:].rearrange("p (n d) -> p n d", d=D), axis=X)
                nc.scalar.activation(out=dist[:, off:off + E], in_=d2[:],
                                     func=Act.Sqrt)

            # ---- mean over all K distances --------------------------
            # column-group split keeps pad slots out of the sums
            nc.vector.reduce_sum(out=parts[:, 7:8], in_=dist[:, 0:LAST_OFF], axis=X)
            nc.vector.reduce_sum(
                out=parts[0:PFULL + 1, 8:9],
                in_=dist[0:PFULL + 1, LAST_OFF:LAST_OFF + PREM], axis=X)
            nc.vector.reduce_sum(
                out=parts[0:PFULL, 9:10],
                in_=dist[0:PFULL, LAST_OFF + PREM:], axis=X)
            # ones-matmul: every PSUM partition gets all three column totals
            psumM = psp.tile([128, 3], f32)
            nc.tensor.matmul(psumM[:], ones[:], parts[:, 7:10])
            mean_bc = pp.tile([128, 1], f32)
            nc.vector.reduce_sum(out=mean_bc[:], in_=psumM[:], axis=X)
            nc.vector.tensor_scalar(
                out=mean_bc[:], in0=mean_bc[:], scalar1=1.0 / K, scalar2=None,
                op0=Alu.mult)

            # ---- nm = relu(mean - dist)^2, hn = count(dist < mean) --
            t = wp.tile([128, KCOLS], f32, bufs=1)
            nc.vector.tensor_scalar(
                out=t[:], in0=dist[:], scalar1=mean_bc[:, 0:1], scalar2=None,
                op0=Alu.subtract)          # t = dist - mean
            nm = wp.tile([128, KCOLS], f32, bufs=1)
            nc.vector.scalar_tensor_tensor(
                out=nm[:], in0=t[:], scalar=0.0, in1=t[:],
                op0=Alu.min, op1=Alu.mult)  # min(t,0)*t == relu(mean-dist)^2
            ind = wp.tile([128, KCOLS], f32, bufs=1)
            nc.vector.tensor_scalar(
                out=ind[:], in0=t[:], scalar1=0.0, scalar2=None,
                op0=Alu.is_lt)
            nc.vector.reduce_sum(out=parts[:, 1:2], in_=nm[:, 0:LAST_OFF], axis=X)
            nc.vector.reduce_sum(
                out=parts[0:PFULL + 1, 2:3],
                in_=nm[0:PFULL + 1, LAST_OFF:LAST_OFF + PREM], axis=X)
            nc.vector.reduce_sum(
                out=parts[0:PFULL, 3:4], in_=nm[0:PFULL, LAST_OFF + PREM:], axis=X)
            nc.vector.reduce_sum(out=parts[:, 4:5], in_=ind[:, 0:LAST_OFF], axis=X)
            nc.vector.reduce_sum(
                out=parts[0:PFULL + 1, 5:6],
                in_=ind[0:PFULL + 1, LAST_OFF:LAST_OFF + PREM], axis=X)
            nc.vector.reduce_sum(
                out=parts[0:PFULL, 6:7], in_=ind[0:PFULL, LAST_OFF + PREM:], axis=X)

            # ---- fold partitions with a second ones-matmul ----------
            psumS = psp.tile([128, PC], f32)
            nc.tensor.matmul(psumS[:], ones[:], parts[:])
            stats_row = pp.tile([1, PC], f32)
            nc.vector.tensor_copy(stats_row[:], psumS[0:1, :])
            nc.sync.dma_start(out=stats[:], in_=stats_row[:])

    nc.finalize()
    return nc


def _get_nc():
    if "nc" not in _CACHE:
        _CACHE["nc"] = _build_nc()
    return _CACHE["nc"]


def _marshal_match(idx):
    """[M] int32 -> [128*EM] row-major [128, EM]; pads (idx 0) fill
    partitions 125..127."""
    buf = np.zeros(128 * EM, dtype=np.int32)
    buf[:M] = idx
    return buf


def _marshal_nonmatch(idx):
    """[K] int32 -> [128*KCOLS] laid out as [128, KCOLS] where chunk c's
    columns hold its 128*E_c indices row-major; pads (idx 0) fall on the
    tail of the last chunk."""
    padded = np.zeros(128 * KCOLS, dtype=np.int32)
    padded[:K] = idx
    full = np.empty((128, KCOLS), dtype=np.int32)
    base = 0
    for off, E in zip(KOFF, KE):
        nblk = 128 * E
        full[:, off:off + E] = padded[base:base + nblk].reshape(128, E)
        base += nblk
    return full.reshape(-1)


def _make_in_maps(outA, outB, matchA, matchB, nonMatchA, nonMatchB):
    outA = np.asarray(outA, dtype=np.float32)
    outB = np.asarray(outB, dtype=np.float32)
    matchA = np.asarray(matchA, dtype=np.int32)
    matchB = np.asarray(matchB, dtype=np.int32)
    nonMatchA = np.asarray(nonMatchA, dtype=np.int32)
    nonMatchB = np.asarray(nonMatchB, dtype=np.int32)
    return [
        {
            "outA": np.ascontiguousarray(outA[b]),
            "outB": np.ascontiguousarray(outB[b]),
            "idxMA": _marshal_match(matchA[b]),
            "idxMB": _marshal_match(matchB[b]),
            "idxNA": _marshal_nonmatch(nonMatchA[b]),
            "idxNB": _marshal_nonmatch(nonMatchB[b]),
        }
        for b in range(B)
    ]


def kernel(outA, outB, matchA, matchB, nonMatchA, nonMatchB, hardNegative):
    from concourse.bass_utils import run_bass_kernel_spmd

    hard = int(np.asarray(hardNegative))
    nc = _get_nc()
    in_maps = _make_in_maps(outA, outB, matchA, matchB, nonMatchA, nonMatchB)
    res = run_bass_kernel_spmd(nc, in_maps, core_ids=list(range(B)))
    stats = np.stack([np.asarray(r["stats"]).reshape(PC) for r in res.results])

    match_sq = stats[:, 0].astype(np.float64)
    nm_sum = (stats[:, 1] + stats[:, 2] + stats[:, 3]).astype(np.float64)
    hn = (stats[:, 4] + stats[:, 5] + stats[:, 6]).astype(np.float64)
    match_loss = (match_sq / M).astype(np.float32)
    if hard:
        denom = np.where(hn == 0, float(K), hn)
    else:
        denom = np.full(B, float(K))
    nm_loss = (NON_MATCH_LOSS_WEIGHT * nm_sum / denom).astype(np.float32)

    match_sum = np.float32(np.sum(match_loss, dtype=np.float32))
    non_match_sum = np.float32(np.sum(nm_loss, dtype=np.float32))
    return (
        np.float32(match_sum + non_match_sum),
        match_sum,
        non_match_sum,
    )


# revision 17
# speedup vs baseline: 26.7376x; 1.0558x over previous
"""Contrastive L2 loss (match/non-match descriptor loss) on Trainium2.

Strategy: data-parallel over batch B=8 across 8 NeuronCores (1 image pair per
core).  Per core the kernel:
  * loads the index tensors into [128, E] SBUF tiles (two HWDGE rings),
  * indirect-DMA gathers the 64B descriptor rows for match (2x5120) and
    non-match (2x50048, 7 chunks/side) straight from the HBM-resident
    outA/outB shards (this random 64B-row gather is the memory roofline),
  * computes sum((mA-mB)^2) (DVE, exact), dist=sqrt(sum((nA-nB)^2, D))
    (DVE squares + ACT sqrt), mean via a ones-matmul on the idle PE (fused
    cross-partition reduce + broadcast through PSUM), then
    nm=relu(mean-dist)^2 sums and the hard-negative count on DVE,
  * folds the per-partition partials with a second ones-matmul and writes
    one [1, PC] stats row to DRAM.
Host combines the 8 stats rows into the 3 reference scalars.

Indirect-DMA contract (validated on HW via a mapping probe, confirmed in the
SWDGE ucode dma_memcopy.cpp): the offset AP must be a [128, E] tile (the Q7
allgather always reads all 128 channels at the AP's per-partition byte
offset); the descriptor fed by idx[p, j] lands at dest partition p, 16-elem
block j.  Max 8192 indices per instruction (Q7 data-scratch: 512 vectors).
Per-instruction Pool time is ~1.1us FLAT in n (measured 1024..8192), so few
big gathers >> many small ones: the previous 125-descriptor-per-instruction
version spent 1.43ms in 1080 instructions; this one spends ~18us in 16.

Padding: gather counts are rounded up to multiples of 128.  Pad slots use
index 0 (safe reads) and are excluded purely by AP slicing with partition
start 0 (engine APs must start on an aligned partition, so no
[127:128, ...]-style accesses):
  * match: 5120 slots = [128, 40]; real 5000 = partitions 0..124 exactly;
    the match reduction reads [0:125, :].
  * non-match: 50048 slots = [128, 391] in 7 chunks (6x64 + 7 cols); the
    first 6 chunks are fully real; the last chunk (cols 384:391) has real
    slots on partitions 0..120 everywhere plus partition 121 in its first
    column.  dist/nm/ind are computed over the full [128, 391] (pad values
    are finite garbage) and every reduction splits into three column
    groups: cols 0:384 x 128p, col 384 x 122p, cols 385:391 x 121p, into
    separate `parts` columns summed in the final fold.
"""

import numpy as np

B, N, D = 8, 307200, 16
M, K = 5000, 50000
NON_MATCH_LOSS_WEIGHT = 1.0

EM = 40                      # match index columns; 128*40 = 5120 >= M
KE = [64, 64, 64, 64, 64, 64, 7]    # non-match chunk widths (cols)
KCOLS = sum(KE)              # 391; 128*391 = 50048 >= K
KOFF = [sum(KE[:i]) for i in range(len(KE))]
LAST_E = KE[-1]
LAST_OFF = KOFF[-1]          # 384
_R = K - 128 * LAST_OFF      # real slots in last chunk: 848
PFULL = _R // LAST_E         # 121 partitions fully real in last chunk
PREM = _R % LAST_E           # 1 extra real col on partition PFULL
assert PREM > 0
# parts columns: 0=match_sq, 1..3=nm_{a,b,c}, 4..6=hn_{a,b,c},
# 7..9=dist_{a,b,c}; groups: a=cols 0:384 x 128p, b=col 384 x (PFULL+1)p,
# c=cols 385:391 x PFULL p
PC = 10

_CACHE = {}


def _build_nc():
    import concourse.bacc as bacc
    import concourse.mybir as mybir
    from concourse.bass import IndirectOffsetOnAxis, MemorySpace
    from concourse.tile import TileContext

    f32 = mybir.dt.float32
    i32 = mybir.dt.int32
    X = mybir.AxisListType.X
    Alu = mybir.AluOpType
    Act = mybir.ActivationFunctionType

    nc = bacc.Bacc()
    outA = nc.declare_dram_parameter("outA", [N, D], f32, isOutput=False)
    outB = nc.declare_dram_parameter("outB", [N, D], f32, isOutput=False)
    idxMA = nc.declare_dram_parameter("idxMA", [128 * EM], i32, isOutput=False)
    idxMB = nc.declare_dram_parameter("idxMB", [128 * EM], i32, isOutput=False)
    idxNA = nc.declare_dram_parameter("idxNA", [128 * KCOLS], i32, isOutput=False)
    idxNB = nc.declare_dram_parameter("idxNB", [128 * KCOLS], i32, isOutput=False)
    stats = nc.declare_dram_parameter("stats", [1, PC], f32, isOutput=True)

    with TileContext(nc) as tc:
        with (
            tc.tile_pool(name="idx", bufs=1) as idxp,
            tc.tile_pool(name="gather", bufs=3) as gp,
            tc.tile_pool(name="work", bufs=2) as wp,
            tc.tile_pool(name="persist", bufs=1) as pp,
            tc.tile_pool(name="psum", bufs=1, space=MemorySpace.PSUM) as psp,
        ):
            # ---- index tiles (split across the two HWDGE rings) -----
            tMA = idxp.tile([128, EM], i32)
            tMB = idxp.tile([128, EM], i32)
            tNA = idxp.tile([128, KCOLS], i32)
            tNB = idxp.tile([128, KCOLS], i32)
            nc.sync.dma_start(out=tMA[:], in_=idxMA[:].rearrange("(p e) -> p e", e=EM))
            nc.scalar.dma_start(out=tMB[:], in_=idxMB[:].rearrange("(p e) -> p e", e=EM))
            nc.sync.dma_start(out=tNA[:], in_=idxNA[:].rearrange("(p e) -> p e", e=KCOLS))
            nc.scalar.dma_start(out=tNB[:], in_=idxNB[:].rearrange("(p e) -> p e", e=KCOLS))

            # ---- persistent state -----------------------------------
            dist = pp.tile([128, KCOLS], f32)
            parts = pp.tile([128, PC], f32)
            ones = pp.tile([128, 128], f32)
            nc.vector.memset(parts[:], 0.0)
            nc.vector.memset(ones[:], 1.0)

            def gather(dst, table, idx_ap):
                nc.gpsimd.indirect_dma_start(
                    out=dst, out_offset=None, in_=table,
                    in_offset=IndirectOffsetOnAxis(ap=idx_ap, axis=0))

            # ---- match part -----------------------------------------
            mA = gp.tile([128, EM * D], f32)
            mB = gp.tile([128, EM * D], f32)
            gather(mA[:], outA[:], tMA[:])
            gather(mB[:], outB[:], tMB[:])
            mD = wp.tile([128, EM * D], f32, bufs=1)
            nc.vector.tensor_sub(mD[:], mA[:], mB[:])
            mSq = wp.tile([128, EM * D], f32, bufs=1)
            nc.vector.tensor_mul(mSq[:], mD[:], mD[:])
            nc.vector.reduce_sum(out=parts[0:125, 0:1], in_=mSq[0:125, :], axis=X)

            # ---- non-match distances, chunked -----------------------
            for c, (off, E) in enumerate(zip(KOFF, KE)):
                nA = gp.tile([128, E * D], f32, tag=f"nA{E}")
                nB = gp.tile([128, E * D], f32, tag=f"nB{E}")
                gather(nA[:], outA[:], tNA[:, off:off + E])
                gather(nB[:], outB[:], tNB[:, off:off + E])
                df = wp.tile([128, E * D], f32, tag=f"df{E}")
                nc.vector.tensor_sub(df[:], nA[:], nB[:])
                sq = wp.tile([128, E * D], f32, tag=f"sq{E}")
                nc.vector.tensor_mul(sq[:], df[:], df[:])
                d2 = wp.tile([128, E], f32, tag=f"d2{E}")
                nc.vector.reduce_sum(
                    out=d2[:], in_=sq[:].rearrange("p (n d) -> p n d", d=D), axis=X)
                nc.scalar.activation(out=dist[:, off:off + E], in_=d2[:],
                                     func=Act.Sqrt)

            # ---- mean over all K distances --------------------------
            # column-group split keeps pad slots out of the sums
            nc.vector.reduce_sum(out=parts[:, 7:8], in_=dist[:, 0:LAST_OFF], axis=X)
            nc.vector.reduce_sum(
                out=parts[0:PFULL + 1, 8:9],
                in_=dist[0:PFULL + 1, LAST_OFF:LAST_OFF + PREM], axis=X)
            nc.vector.reduce_sum(
                out=parts[0:PFULL, 9:10],
                in_=dist[0:PFULL, LAST_OFF + PREM:], axis=X)
            # ones-matmul: every PSUM partition gets all three column totals
            psumM = psp.tile([128, 3], f32)
            nc.tensor.matmul(psumM[:], ones[:], parts[:, 7:10])
            mean_bc = pp.tile([128, 1], f32)
            nc.vector.reduce_sum(out=mean_bc[:], in_=psumM[:], axis=X)
            nc.vector.tensor_scalar(
                out=mean_bc[:], in0=mean_bc[:], scalar1=1.0 / K, scalar2=None,
                op0=Alu.mult)

            # ---- nm = relu(mean - dist)^2, hn = count(dist < mean) --
            t = wp.tile([128, KCOLS], f32, bufs=1)
            nc.vector.tensor_scalar(
                out=t[:], in0=dist[:], scalar1=mean_bc[:, 0:1], scalar2=None,
                op0=Alu.subtract)          # t = dist - mean
            nm = wp.tile([128, KCOLS], f32, bufs=1)
            nc.vector.scalar_tensor_tensor(
                out=nm[:], in0=t[:], scalar=0.0, in1=t[:],
                op0=Alu.min, op1=Alu.mult)  # min(t,0)*t == relu(mean-dist)^2
            ind = wp.tile([128, KCOLS], f32, bufs=1)
            nc.vector.tensor_scalar(
                out=ind[:], in0=t[:], scalar1=0.0, scalar2=None,
                op0=Alu.is_lt)
            nc.vector.reduce_sum(out=parts[:, 1:2], in_=nm[:, 0:LAST_OFF], axis=X)
            nc.vector.reduce_sum(
                out=parts[0:PFULL + 1, 2:3],
                in_=nm[0:PFULL + 1, LAST_OFF:LAST_OFF + PREM], axis=X)
            nc.vector.reduce_sum(
                out=parts[0:PFULL, 3:4], in_=nm[0:PFULL, LAST_OFF + PREM:], axis=X)
            nc.vector.reduce_sum(out=parts[:, 4:5], in_=ind[:, 0:LAST_OFF], axis=X)
            nc.vector.reduce_sum(
                out=parts[0:PFULL + 1, 5:6],
                in_=ind[0:PFULL + 1, LAST_OFF:LAST_OFF + PREM], axis=X)
            nc.vector.reduce_sum(
                out=parts[0:PFULL, 6:7], in_=ind[0:PFULL, LAST_OFF + PREM:], axis=X)

            # ---- fold partitions with a second ones-matmul ----------
            psumS = psp.tile([128, PC], f32)
            nc.tensor.matmul(psumS[:], ones[:], parts[:])
            stats_row = pp.tile([1, PC], f32)
            nc.vector.tensor_copy(stats_row[:], psumS[0:1, :])
            nc.sync.dma_start(out=stats[:], in_=stats_row[:])

    nc.finalize()
    return nc


def _get_nc():
    if "nc" not in _CACHE:
        _CACHE["nc"] = _build_nc()
    return _CACHE["nc"]


def _marshal_match(idx):
    """[M] int32 -> [128*EM] row-major [128, EM]; pads (idx 0) fill
    partitions 125..127."""
    buf = np.zeros(128 * EM, dtype=np.int32)
    buf[:M] = idx
    return buf


def _marshal_nonmatch(idx):
    """[K] int32 -> [128*KCOLS] laid out as [128, KCOLS] where chunk c's
    columns hold its 128*E_c indices row-major; pads (idx 0) fall on the
    tail of the last chunk."""
    padded = np.zeros(128 * KCOLS, dtype=np.int32)
    padded[:K] = idx
    full = np.empty((128, KCOLS), dtype=np.int32)
    base = 0
    for off, E in zip(KOFF, KE):
        nblk = 128 * E
        full[:, off:off + E] = padded[base:base + nblk].reshape(128, E)
        base += nblk
    return full.reshape(-1)


def _make_in_maps(outA, outB, matchA, matchB, nonMatchA, nonMatchB):
    outA = np.asarray(outA, dtype=np.float32)
    outB = np.asarray(outB, dtype=np.float32)
    matchA = np.asarray(matchA, dtype=np.int32)
    matchB = np.asarray(matchB, dtype=np.int32)
    nonMatchA = np.asarray(nonMatchA, dtype=np.int32)
    nonMatchB = np.asarray(nonMatchB, dtype=np.int32)
    return [
        {
            "outA": np.ascontiguousarray(outA[b]),
            "outB": np.ascontiguousarray(outB[b]),
            "idxMA": _marshal_match(matchA[b]),
            "idxMB": _marshal_match(matchB[b]),
            "idxNA": _marshal_nonmatch(nonMatchA[b]),
            "idxNB": _marshal_nonmatch(nonMatchB[b]),
        }
        for b in range(B)
    ]


def kernel(outA, outB, matchA, matchB, nonMatchA, nonMatchB, hardNegative):
    from concourse.bass_utils import run_bass_kernel_spmd

    hard = int(np.asarray(hardNegative))
    nc = _get_nc()
    in_maps = _make_in_maps(outA, outB, matchA, matchB, nonMatchA, nonMatchB)
    res = run_bass_kernel_spmd(nc, in_maps, core_ids=list(range(B)))
    stats = np.stack([np.asarray(r["stats"]).reshape(PC) for r in res.results])

    match_sq = stats[:, 0].astype(np.float64)
    nm_sum = (stats[:, 1] + stats[:, 2] + stats[:, 3]).astype(np.float64)
    hn = (stats[:, 4] + stats[:, 5] + stats[:, 6]).astype(np.float64)
    match_loss = (match_sq / M).astype(np.float32)
    if hard:
        denom = np.where(hn == 0, float(K), hn)
    else:
        denom = np.full(B, float(K))
    nm_loss = (NON_MATCH_LOSS_WEIGHT * nm_sum / denom).astype(np.float32)

    match_sum = np.float32(np.sum(match_loss, dtype=np.float32))
    non_match_sum = np.float32(np.sum(nm_loss, dtype=np.float32))
    return (
        np.float32(match_sum + non_match_sum),
        match_sum,
        non_match_sum,
    )
